# revision 1
# baseline (speedup 1.0000x reference)
"""DiGCN (2-layer GCNConv + parallel Linear + BatchNorm1d + ReLU) on 8 trn2 NeuronCores.

Strategy (matches the problem's sharding hint):
  - Shard nodes contiguously across 8 cores (12500 nodes/core), replicate the
    small [D,D] weights, partition edges by destination-node owner.
  - Per layer: each core computes hg = h_local @ gcn_w.T (bf16), AllGathers hg
    so every core holds the full [N,D] table; gathers its edges' source rows
    via the GPSIMD dma_gather extended instruction; scatter-adds into its local
    destination nodes via one-hot matmuls on the TensorEngine (edges sorted by
    dest tile; a host-built [128e x 128d] one-hot*norm bf16 matrix per 128-edge
    chunk turns segment-sum into PSUM accumulation). BN stats via AllReduce.
  - dma_gather indices are int16, so the [N,D] table is addressed through
    range buckets of 32767 rows; slots are laid out bucket-major per batch so
    each (batch, bucket) is one packed gather call.
  - h is kept transposed in SBUF ([128 feat, nodes], f32) so BN reduction is a
    free-dim reduce and BN+ReLU fuse into one ScalarE activation pass.

kernel(**inputs) takes FULL inputs, returns the FULL [N,D] float32 output.
"""

import math
import os
import sys

import numpy as np

for _p in ("/opt/trn_rl_repo", "/root/.axon_site/_ro/trn_rl_repo"):
    if os.path.isdir(_p) and _p not in sys.path:
        sys.path.insert(0, _p)

# ---------------------------------------------------------------- configuration
N_GLOBAL = 100000
E_GLOBAL = 500000
D = 128
DEPTH = 2
EPS = 1e-5
NCORES = 8
IDXMAX = 32767  # dma_gather int16 index limit (rows per range bucket)

LAST_RUNNER = None  # (run_once, fetch) of the most recent kernel() call


class _Cfg:
    def __init__(self, n_global, ncores, depth=DEPTH, eps=EPS, idxmax=IDXMAX, tb=8):
        assert n_global % ncores == 0
        self.n_global = n_global
        self.ncores = ncores
        self.depth = depth
        self.eps = eps
        self.idxmax = idxmax
        self.tb = tb                                # dest tiles per gather batch
        self.np_local = n_global // ncores          # real nodes per core
        self.nt = math.ceil(self.np_local / 128)    # dest tiles per core
        self.npad = self.nt * 128                   # padded nodes per core
        self.nb = math.ceil(n_global / idxmax)      # gather range buckets
        self.bases = [b * idxmax for b in range(self.nb)]


def _layout(cfg, K):
    """Chunk/call enumeration shared by host prep and the bass builder.

    Returns (chunk_meta, call_meta, cidx, batches):
      chunk_meta[c] = (bucket, tile, j)
      call_meta = list of (batch_idx, bucket, c0, nchunks_in_call)
      cidx[(t, b, j)] = global chunk index
      batches = list of (t0, t1, bc0, bc1)  (tile range, chunk range)
    """
    chunk_meta, call_meta, cidx, batches = [], [], {}, []
    nbatch = math.ceil(cfg.nt / cfg.tb)
    for bi in range(nbatch):
        t0, t1 = bi * cfg.tb, min(cfg.nt, bi * cfg.tb + cfg.tb)
        bc0 = len(chunk_meta)
        for b in range(cfg.nb):
            if K[b] == 0:
                continue
            c0 = len(chunk_meta)
            for t in range(t0, t1):
                for j in range(K[b]):
                    cidx[(t, b, j)] = len(chunk_meta)
                    chunk_meta.append((b, t, j))
            call_meta.append((bi, b, c0, len(chunk_meta) - c0))
        batches.append((t0, t1, bc0, len(chunk_meta)))
    return chunk_meta, call_meta, cidx, batches


# ---------------------------------------------------------------- host-side prep
def _prep_graph(cfg, edge_index, edge_weight):
    """Partition edges by destination owner/tile/src-bucket; build int16 gather
    indices and the per-chunk one-hot*norm matrices (bf16)."""
    import ml_dtypes

    row = np.asarray(edge_index[0], dtype=np.int64)
    col = np.asarray(edge_index[1], dtype=np.int64)
    w = np.asarray(edge_weight, dtype=np.float32)
    n, nb, nt, npl = cfg.n_global, cfg.nb, cfg.nt, cfg.np_local

    deg = np.bincount(col, weights=w.astype(np.float64), minlength=n).astype(np.float32)
    dinv = np.where(deg > 0, 1.0 / np.sqrt(np.where(deg > 0, deg, 1.0)), 0.0).astype(
        np.float32
    )
    norm = (dinv[row] * w * dinv[col]).astype(np.float32)

    core = col // npl
    lc = col % npl
    tile = lc // 128
    d_in_tile = lc % 128
    bucket = np.minimum(row // cfg.idxmax, nb - 1)

    key = (core * nt + tile) * nb + bucket
    order = np.argsort(key, kind="stable")
    key_s = key[order]
    counts = np.bincount(key, minlength=cfg.ncores * nt * nb)
    starts = np.concatenate([[0], np.cumsum(counts)[:-1]])
    rank = np.arange(row.shape[0], dtype=np.int64) - starts[key_s]

    cnt3 = counts.reshape(cfg.ncores, nt, nb)
    K = [int(math.ceil(cnt3[:, :, b].max() / 128)) for b in range(nb)]

    chunk_meta, call_meta, cidx, batches = _layout(cfg, K)
    nchunks = len(chunk_meta)

    # per-edge slot: chunk c = cidx[(tile, bucket, rank//128)], partition rank%128
    cidx_arr = np.full((nt, nb, max(K)), -1, dtype=np.int64)
    for (t, b, j), c in cidx.items():
        cidx_arr[t, b, j] = c
    t_s = (key_s // nb) % nt
    b_s = key_s % nb
    core_s = key_s // (nb * nt)
    j_s = rank // 128
    p_s = rank % 128
    c_s = cidx_arr[t_s, b_s, j_s]
    assert (c_s >= 0).all()

    base_arr = np.asarray(cfg.bases, dtype=np.int64)
    val_s = (row[order] - base_arr[b_s]).astype(np.int16)
    assert (val_s >= 0).all()

    # int16 gather indices: element i of a call lives at [i%16, chunk*8 + p//16]
    idx_all = np.zeros((cfg.ncores, 128, nchunks * 8), dtype=np.int16)
    idx_all[core_s, p_s % 16, c_s * 8 + p_s // 16] = val_s
    idx_all[:, 16:32, :] = idx_all[:, 0:16, :]  # replica for the 2nd Q7 cpu

    mt_all = np.zeros((cfg.ncores, 128, nchunks * 128), dtype=ml_dtypes.bfloat16)
    mt_all[core_s, p_s, c_s * 128 + d_in_tile[order]] = norm[order].astype(
        ml_dtypes.bfloat16
    )
    return K, idx_all, mt_all


def _prep_inputs(cfg, K, idx_all, mt_all, x, lin_w, gcn_w, gamma, beta):
    x = np.asarray(x, dtype=np.float32)
    npl, npad = cfg.np_local, cfg.npad
    wlin = np.concatenate([lin_w[i].T for i in range(cfg.depth)], axis=1).astype(
        np.float32
    )  # [D, depth*D], column block i = lin_w[i].T  (k, o)
    wgcn = np.concatenate([gcn_w[i].T for i in range(cfg.depth)], axis=1).astype(
        np.float32
    )
    gb = np.stack(
        sum([[gamma[i], beta[i]] for i in range(cfg.depth)], []), axis=1
    ).astype(np.float32)  # [D, 2*depth]: columns g0,b0,g1,b1

    in_maps = []
    for r in range(cfg.ncores):
        xs = x[r * npl : (r + 1) * npl]
        xT = np.zeros((D, npad), dtype=np.float32)
        xT[:, :npl] = xs.T
        in_maps.append(
            {
                "xT": np.ascontiguousarray(xT),
                "wlin": np.ascontiguousarray(wlin),
                "wgcn": np.ascontiguousarray(wgcn),
                "gb": np.ascontiguousarray(gb),
                "gidx": np.ascontiguousarray(idx_all[r]),
                "mt": np.ascontiguousarray(mt_all[r]),
            }
        )
    return in_maps


# ---------------------------------------------------------------- bass program
def _build_program(cfg, K):
    from concourse import bacc, mybir, tile

    f32 = mybir.dt.float32
    bf16 = mybir.dt.bfloat16
    i16 = mybir.dt.int16
    npl, npad, nt, nb = cfg.np_local, cfg.npad, cfg.nt, cfg.nb
    rg = [list(range(cfg.ncores))]
    inv_n = 1.0 / cfg.n_global
    skips = set(os.environ.get("KERNEL_SKIP", "").split(","))
    maxcalls = int(os.environ.get("KERNEL_MAXCALLS", "999999"))
    ncalls_done = 0

    chunk_meta, call_meta, cidx, batches = _layout(cfg, K)
    nchunks = len(chunk_meta)
    ktot = sum(K)
    bseq = [(b, j) for b in range(nb) if K[b] > 0 for j in range(K[b])]
    gcols = cfg.tb * ktot * 128  # max chunk-columns per batch buffer

    nc = bacc.Bacc(
        "TRN2", target_bir_lowering=False, debug=False, num_devices=cfg.ncores
    )

    xT = nc.dram_tensor("xT", [D, npad], f32, kind="ExternalInput")
    wlin_d = nc.dram_tensor("wlin", [D, cfg.depth * D], f32, kind="ExternalInput")
    wgcn_d = nc.dram_tensor("wgcn", [D, cfg.depth * D], f32, kind="ExternalInput")
    gb_d = nc.dram_tensor("gb", [D, 2 * cfg.depth], f32, kind="ExternalInput")
    gidx_d = nc.dram_tensor("gidx", [128, nchunks * 8], i16, kind="ExternalInput")
    mt_d = nc.dram_tensor("mt", [128, nchunks * 128], bf16, kind="ExternalInput")
    outT_d = nc.dram_tensor("outT", [D, npl], f32, kind="ExternalOutput")

    with tile.TileContext(nc) as tc:
        with (
            tc.tile_pool(name="big", bufs=1) as big,
            tc.tile_pool(name="gpool", bufs=2) as gpool,
            tc.tile_pool(name="mpool", bufs=2) as mpool,
            tc.tile_pool(name="ipool", bufs=2) as ipool,
            tc.tile_pool(name="cpool", bufs=4) as cpool,
            tc.tile_pool(name="small", bufs=1) as small,
            tc.tile_pool(name="stats", bufs=2) as stats_pool,
            tc.tile_pool(name="psA", bufs=4, space="PSUM") as psA,
            tc.tile_pool(name="psH", bufs=2, space="PSUM") as psH,
            tc.tile_pool(name="dram", bufs=1, space="DRAM") as dpool,
        ):
            hA = big.tile([128, npad], f32)
            hB = big.tile([128, npad], f32)
            consts = small.tile([128, 2], f32)  # col0 = 0.0, col1 = eps
            nc.vector.memset(consts[:, 0:1], 0.0)
            nc.vector.memset(consts[:, 1:2], float(cfg.eps))
            wlin = small.tile([128, cfg.depth * D], f32)
            wgcn = small.tile([128, cfg.depth * D], f32)
            gb = small.tile([128, 2 * cfg.depth], f32)

            nc.sync.dma_start(out=hA[:, :], in_=xT[:, :])
            nc.sync.dma_start(out=wlin[:, :], in_=wlin_d[:, :])
            nc.sync.dma_start(out=wgcn[:, :], in_=wgcn_d[:, :])
            nc.sync.dma_start(out=gb[:, :], in_=gb_d[:, :])

            cur, nxt = hA, hB
            for layer in range(cfg.depth):
                # ---- phase A: hg = h @ gcn_w.T (node-major, bf16) -> cc_in
                cc_in = dpool.tile([npl, D], bf16, name=f"cc_in_{layer}")
                cc_out = dpool.tile(
                    [cfg.n_global, D], bf16, addr_space="Shared", name=f"cc_out_{layer}"
                )
                wg = wgcn[:, layer * D : (layer + 1) * D]
                GA = 8  # dest tiles per store group (one wide DMA each)
                for t0a in range(0, nt, GA):
                    t1a = min(nt, t0a + GA)
                    strip = cpool.tile(
                        [128, GA * 128], bf16, name="sb_hg", tag="sb_hg"
                    )
                    for t in range(t0a, t1a):
                        ps = psA.tile([128, 128], f32, name="ps_hg", tag="ps_hg")
                        nc.tensor.matmul(
                            ps[:, :],
                            lhsT=cur[:, t * 128 : (t + 1) * 128],
                            rhs=wg,
                            start=True,
                            stop=True,
                        )
                        nc.vector.tensor_copy(
                            strip[:, (t - t0a) * 128 : (t - t0a + 1) * 128], ps[:, :]
                        )
                    if "astore" in skips:
                        continue
                    n0 = t0a * 128
                    nfull = (min(npl, t1a * 128) - n0) // 128  # whole 128-row tiles
                    if nfull > 0:
                        nc.sync.dma_start(
                            out=cc_in[n0 : n0 + nfull * 128, :].rearrange(
                                "(t p) e -> p t e", p=128
                            ),
                            in_=strip[:, : nfull * 128].rearrange(
                                "p (t e) -> p t e", e=128
                            ),
                        )
                    rem = min(npl, t1a * 128) - (n0 + nfull * 128)
                    if rem > 0:
                        nc.sync.dma_start(
                            out=cc_in[n0 + nfull * 128 :, :],
                            in_=strip[:rem, nfull * 128 : (nfull + 1) * 128],
                        )

                # ---- phase B: AllGather hg
                if "ag" not in skips:
                    nc.gpsimd.collective_compute(
                        "AllGather",
                        mybir.AluOpType.bypass,
                        replica_groups=rg,
                        ins=[cc_in[:, :].opt()],
                        outs=[cc_out[:, :].opt()],
                    )

                # ---- phase C: hl = h @ lin_w.T  (transposed layout, into nxt)
                wl = wlin[:, layer * D : (layer + 1) * D]
                if "hl" in skips:
                    nc.vector.memset(nxt[:, :], 0.0)
                c0 = 0
                while c0 < npad and "hl" not in skips:
                    cw = min(512, npad - c0)
                    ps = psH.tile([128, 512], f32, name="ps_hl", tag="ps_hl")
                    nc.tensor.matmul(
                        ps[:, :cw],
                        lhsT=wl,
                        rhs=cur[:, c0 : c0 + cw],
                        start=True,
                        stop=True,
                    )
                    nc.vector.tensor_copy(nxt[:, c0 : c0 + cw], ps[:, :cw])
                    c0 += cw

                # ---- phase D: bucketed dma_gather + one-hot matmul scatter-add
                for t0, t1, bc0, bc1 in batches:
                    nch_b = bc1 - bc0
                    g = gpool.tile([128, gcols], bf16, name="gbuf", tag="gbuf")
                    m = mpool.tile([128, gcols], bf16, name="mbuf", tag="mbuf")
                    wb = ipool.tile([128, cfg.tb * ktot * 8], i16, name="wb", tag="wb")
                    nc.sync.dma_start(
                        out=wb[:, : nch_b * 8], in_=gidx_d[:, bc0 * 8 : bc1 * 8]
                    )
                    if "mt" not in skips:
                        nc.sync.dma_start(
                            out=m[:, : nch_b * 128],
                            in_=mt_d[:, bc0 * 128 : bc1 * 128],
                        )
                    if "gather" not in skips:
                        for bi2, b, c0g, ncall in call_meta:
                            if not (bc0 <= c0g < bc1):
                                continue
                            if ncalls_done >= maxcalls:
                                continue
                            ncalls_done += 1
                            cl0 = c0g - bc0
                            out3 = g[
                                :, cl0 * 128 : (cl0 + ncall) * 128
                            ].rearrange("p (c e) -> p c e", e=128)
                            nc.gpsimd.dma_gather(
                                out3,
                                cc_out[cfg.bases[b] :, :],
                                wb[:, cl0 * 8 : (cl0 + ncall) * 8],
                                ncall * 128,
                                ncall * 128,
                                128,
                                single_packet=False,
                            )
                    if "aggmm" in skips:
                        continue
                    for t in range(t0, t1):
                        ps = psA.tile([128, 128], f32, name="ps_agg", tag="ps_hg")
                        for si, (b, j) in enumerate(bseq):
                            cl = cidx[(t, b, j)] - bc0
                            nc.tensor.matmul(
                                ps[:, :],
                                lhsT=g[:, cl * 128 : (cl + 1) * 128],
                                rhs=m[:, cl * 128 : (cl + 1) * 128],
                                start=(si == 0),
                                stop=(si == len(bseq) - 1),
                            )
                        nc.vector.tensor_add(
                            nxt[:, t * 128 : (t + 1) * 128],
                            nxt[:, t * 128 : (t + 1) * 128],
                            ps[:, :],
                        )

                # ---- phase E: BatchNorm stats + AllReduce
                skip_bn = "bn" in skips
                st = stats_pool.tile([128, 2], f32, name=f"st_{layer}")
                if not skip_bn:
                    nc.vector.reduce_sum(
                        out=st[:, 0:1], in_=nxt[:, :npl], axis=mybir.AxisListType.X
                    )
                if not skip_bn:
                    nc.scalar.activation(
                        out=cur[:, :npl],
                        in_=nxt[:, :npl],
                        func=mybir.ActivationFunctionType.Square,
                        bias=consts[:, 0:1],
                        accum_out=st[:, 1:2],
                    )
                if skip_bn:
                    nc.vector.memset(st[:, :], 1.0)
                bn_in = dpool.tile([128, 2], f32, name=f"bn_in_{layer}")
                bn_out = dpool.tile(
                    [128, 2], f32, addr_space="Shared", name=f"bn_out_{layer}"
                )
                nc.sync.dma_start(out=bn_in[:, :], in_=st[:, :])
                if "ar" not in skips:
                    nc.gpsimd.collective_compute(
                        "AllReduce",
                        mybir.AluOpType.add,
                        replica_groups=rg,
                        ins=[bn_in[:, :].opt()],
                        outs=[bn_out[:, :].opt()],
                    )
                gst = stats_pool.tile([128, 2], f32, name=f"gst_{layer}")
                nc.sync.dma_start(out=gst[:, :], in_=bn_out[:, :])

                # scale = gamma * rsqrt(var+eps); bias = beta - mean*scale
                mu = stats_pool.tile([128, 1], f32, name=f"mu_{layer}")
                vr = stats_pool.tile([128, 1], f32, name=f"vr_{layer}")
                sc = stats_pool.tile([128, 1], f32, name=f"sc_{layer}")
                bi = stats_pool.tile([128, 1], f32, name=f"bi_{layer}")
                nc.scalar.mul(mu[:, :], gst[:, 0:1], inv_n)  # mean
                nc.vector.tensor_scalar(
                    out=vr[:, :],
                    in0=gst[:, 1:2],
                    scalar1=inv_n,
                    scalar2=None,
                    op0=mybir.AluOpType.mult,
                )
                mu2 = stats_pool.tile([128, 1], f32, name=f"mu2_{layer}")
                nc.scalar.activation(
                    out=mu2[:, :],
                    in_=mu[:, :],
                    func=mybir.ActivationFunctionType.Square,
                    bias=consts[:, 0:1],
                )
                nc.vector.tensor_sub(vr[:, :], vr[:, :], mu2[:, :])
                nc.scalar.activation(
                    out=vr[:, :],
                    in_=vr[:, :],
                    func=mybir.ActivationFunctionType.Sqrt,
                    bias=consts[:, 1:2],
                )
                nc.vector.reciprocal(vr[:, :], vr[:, :])  # rstd
                nc.vector.tensor_mul(sc[:, :], vr[:, :], gb[:, 2 * layer : 2 * layer + 1])
                nc.vector.tensor_mul(bi[:, :], mu[:, :], sc[:, :])
                nc.vector.tensor_sub(bi[:, :], gb[:, 2 * layer + 1 : 2 * layer + 2], bi[:, :])

                # ---- phase F: apply BN (+ReLU except last layer), into cur
                if "apply" in skips:
                    if layer == cfg.depth - 1:
                        nc.sync.dma_start(out=outT_d[:, :], in_=nxt[:, :npl])
                    continue
                func = (
                    mybir.ActivationFunctionType.Relu
                    if layer != cfg.depth - 1
                    else mybir.ActivationFunctionType.Identity
                )
                nc.scalar.activation(
                    out=cur[:, :],
                    in_=nxt[:, :],
                    func=func,
                    bias=bi[:, :],
                    scale=sc[:, :],
                )
                # cur now holds the layer output (transposed); nxt is free
                if layer == cfg.depth - 1:
                    nc.sync.dma_start(out=outT_d[:, :], in_=cur[:, :npl])

    nc.compile()
    return nc


# ---------------------------------------------------------------- entry points
def _make_runner(cfg, nc, in_maps):
    """Build a repeat-callable PJRT runner with device-resident inputs (no
    donation) for wall-clock timing. Returns (run_once, fetch_results)."""
    import jax
    from jax.experimental.shard_map import shard_map
    from jax.sharding import Mesh, NamedSharding, PartitionSpec

    from concourse import bass2jax, mybir

    bass2jax.install_neuronx_cc_hook()

    partition_name = nc.partition_id_tensor.name if nc.partition_id_tensor else None
    in_names, out_names, out_avals, zero_outs = [], [], [], []
    for alloc in nc.m.functions[0].allocations:
        if not isinstance(alloc, mybir.MemoryLocationSet):
            continue
        name = alloc.memorylocations[0].name
        if alloc.kind == "ExternalInput":
            if name != partition_name:
                in_names.append(name)
        elif alloc.kind == "ExternalOutput":
            out_names.append(name)
            shape = tuple(alloc.tensor_shape)
            dtype = mybir.dt.np(alloc.dtype)
            out_avals.append(jax.core.ShapedArray(shape, dtype))
            zero_outs.append(np.zeros(shape, dtype))
    n_params = len(in_names)
    all_in_names = list(in_names) + list(out_names)
    if partition_name is not None:
        all_in_names.append(partition_name)

    def _body(*args):
        operands = list(args)
        if partition_name is not None:
            operands.append(bass2jax.partition_id_tensor())
        outs = bass2jax._bass_exec_p.bind(
            *operands,
            out_avals=tuple(out_avals),
            in_names=tuple(all_in_names),
            out_names=tuple(out_names),
            lowering_input_output_aliases=(),
            sim_require_finite=True,
            sim_require_nnan=True,
            nc=nc,
        )
        return tuple(outs)

    n = cfg.ncores
    devices = jax.devices()[:n]
    mesh = Mesh(np.asarray(devices), ("core",))
    n_outs = len(out_names)
    in_specs = (PartitionSpec("core"),) * (n_params + n_outs)
    out_specs = (PartitionSpec("core"),) * n_outs
    sharded = jax.jit(
        shard_map(
            _body, mesh=mesh, in_specs=in_specs, out_specs=out_specs, check_rep=False
        ),
        keep_unused=True,
    )
    shd = NamedSharding(mesh, PartitionSpec("core"))
    concat_in = [
        jax.device_put(
            np.concatenate([np.asarray(in_maps[c][k]) for c in range(n)], axis=0), shd
        )
        for k in in_names
    ]
    concat_zeros = [
        jax.device_put(np.zeros((n * z.shape[0], *z.shape[1:]), z.dtype), shd)
        for z in zero_outs
    ]

    def run_once():
        outs = sharded(*concat_in, *concat_zeros)
        jax.block_until_ready(outs)
        return outs

    def fetch(outs):
        return [
            {
                k: np.asarray(outs[i]).reshape(n, *out_avals[i].shape)[c]
                for i, k in enumerate(out_names)
            }
            for c in range(n)
        ]

    return run_once, fetch


def _assemble(cfg, results):
    out = np.empty((cfg.n_global, D), dtype=np.float32)
    npl = cfg.np_local
    for r in range(cfg.ncores):
        out[r * npl : (r + 1) * npl] = results[r]["outT"].T
    return out


def kernel(x, edge_index, edge_weight, lin_w, gcn_w, gamma, beta):
    global LAST_RUNNER
    cfg = _Cfg(N_GLOBAL, NCORES)
    x = np.asarray(x)
    assert x.shape == (cfg.n_global, D)
    K, idx_all, mt_all = _prep_graph(cfg, np.asarray(edge_index), np.asarray(edge_weight))
    in_maps = _prep_inputs(
        cfg, K, idx_all, mt_all, x, np.asarray(lin_w), np.asarray(gcn_w),
        np.asarray(gamma), np.asarray(beta),
    )
    nc = _build_program(cfg, K)
    run_once, fetch = _make_runner(cfg, nc, in_maps)
    LAST_RUNNER = (run_once, fetch)
    results = fetch(run_once())
    return _assemble(cfg, results)



# revision 58
# speedup vs baseline: 2.8203x; 2.8203x over previous
"""DiGCN (2-layer GCNConv + parallel Linear + BatchNorm1d + ReLU) on 8 trn2 NeuronCores.

Strategy (matches the problem's sharding hint):
  - Shard nodes contiguously across 8 cores (12500 nodes/core), replicate the
    small [D,D] weights, partition edges by destination-node owner.
  - Per layer: each core computes hg = h_local @ gcn_w.T (bf16), AllGathers hg
    so every core holds the full [N,D] table; gathers its edges' source rows
    via the GPSIMD dma_gather extended instruction; scatter-adds into its local
    destination nodes via one-hot matmuls on the TensorEngine (edges sorted by
    dest tile; a host-built [128e x 128d] one-hot*norm bf16 matrix per 128-edge
    chunk turns segment-sum into PSUM accumulation). BN stats via AllReduce.
  - dma_gather indices are int16, so the [N,D] table is addressed through
    range buckets of 32767 rows; slots are laid out bucket-major per batch so
    each (batch, bucket) is one packed gather call.
  - h is kept transposed in SBUF ([128 feat, nodes], f32) so BN reduction is a
    free-dim reduce and BN+ReLU fuse into one ScalarE activation pass.

kernel(**inputs) takes FULL inputs, returns the FULL [N,D] float32 output.
"""

import math
import os
import sys

import numpy as np

for _p in ("/opt/trn_rl_repo", "/root/.axon_site/_ro/trn_rl_repo"):
    if os.path.isdir(_p) and _p not in sys.path:
        sys.path.insert(0, _p)

# ---------------------------------------------------------------- configuration
N_GLOBAL = 100000
E_GLOBAL = 500000
D = 128
DEPTH = 2
EPS = 1e-5
NCORES = 8
IDXMAX = 32767  # dma_gather int16 index limit (rows per range bucket)

LAST_RUNNER = None  # (run_once, fetch) of the most recent kernel() call


class _Cfg:
    def __init__(self, n_global, ncores, depth=DEPTH, eps=EPS, idxmax=IDXMAX,
                 tb=int(os.environ.get("KERNEL_TB", "8"))):
        assert n_global % ncores == 0
        self.n_global = n_global
        self.ncores = ncores
        self.depth = depth
        self.eps = eps
        self.idxmax = idxmax
        self.tb = tb                                # dest tiles per gather batch
        self.np_local = n_global // ncores          # real nodes per core
        self.nt = math.ceil(self.np_local / 128)    # dest tiles per core
        self.npad = self.nt * 128                   # padded nodes per core
        # the hg table is AllGathered in two halves (rows [0, npl/2) and
        # [npl/2, npl) of every core); each half-table has 50000 rows indexed
        # through 2 int16 windows: [0, 32767) and [32767, 50000).
        self.half = self.np_local // 2              # rows per half per core
        self.nhalf = self.half * ncores             # rows per half-table
        self.ngroup = 4                             # (half, window) groups
        self.wbase = [0, idxmax]                    # window bases in half-table


def _layout(cfg, cvar):
    """Chunk/call enumeration shared by host prep and the bass builder.

    cvar: [nt, ngroup] int array — chunks per (tile, group), shared across
    cores (max over cores).

    Returns (chunk_meta, call_meta, cidx, batches):
      chunk_meta[c] = (group, tile, j)
      call_meta = list of (batch_idx, group, c0, nchunks_in_call)
      cidx[(t, g, j)] = global chunk index
      batches = list of (t0, t1, bc0, bc1)  (tile range, chunk range)
    """
    chunk_meta, call_meta, cidx, batches = [], [], {}, []
    nbatch = math.ceil(cfg.nt / cfg.tb)
    for bi in range(nbatch):
        t0, t1 = bi * cfg.tb, min(cfg.nt, bi * cfg.tb + cfg.tb)
        bc0 = len(chunk_meta)
        for g in range(cfg.ngroup):
            c0 = len(chunk_meta)
            for t in range(t0, t1):
                for j in range(int(cvar[t, g])):
                    cidx[(t, g, j)] = len(chunk_meta)
                    chunk_meta.append((g, t, j))
            if len(chunk_meta) > c0:
                call_meta.append((bi, g, c0, len(chunk_meta) - c0))
        batches.append((t0, t1, bc0, len(chunk_meta)))
    return chunk_meta, call_meta, cidx, batches


# ---------------------------------------------------------------- host-side prep
def _prep_graph(cfg, edge_index, edge_weight):
    """Partition edges by destination owner/tile/src-bucket; build int16 gather
    indices and the per-chunk one-hot*norm matrices (bf16)."""
    import ml_dtypes

    row = np.asarray(edge_index[0], dtype=np.int64)
    col = np.asarray(edge_index[1], dtype=np.int64)
    w = np.asarray(edge_weight, dtype=np.float32)
    n, nt, npl = cfg.n_global, cfg.nt, cfg.np_local

    deg = np.bincount(col, weights=w.astype(np.float64), minlength=n).astype(np.float32)
    dinv = np.where(deg > 0, 1.0 / np.sqrt(np.where(deg > 0, deg, 1.0)), 0.0).astype(
        np.float32
    )
    norm = (dinv[row] * w * dinv[col]).astype(np.float32)

    core = col // npl
    lc = col % npl
    tile = lc // 128
    d_in_tile = lc % 128
    # source location in the half-split AllGather tables
    src_core = row // npl
    src_l = row % npl
    src_h = src_l // cfg.half
    row_h = src_core * cfg.half + (src_l % cfg.half)  # row in half-table
    win = np.minimum(row_h // cfg.idxmax, 1)
    grp = src_h * 2 + win
    ng = cfg.ngroup

    key = (core * nt + tile) * ng + grp
    if os.environ.get("KERNEL_SORTSRC", "1") == "1":
        # secondary sort by source row: descriptors within a chunk sweep the
        # gather table near-monotonically (HBM row-buffer locality)
        order = np.lexsort((row_h, key))
    else:
        order = np.argsort(key, kind="stable")
    key_s = key[order]
    counts = np.bincount(key, minlength=cfg.ncores * nt * ng)
    starts = np.concatenate([[0], np.cumsum(counts)[:-1]])
    rank = np.arange(row.shape[0], dtype=np.int64) - starts[key_s]

    cnt3 = counts.reshape(cfg.ncores, nt, ng)
    cvar = np.ceil(cnt3.max(axis=0) / 128).astype(np.int64)  # [nt, ng]

    chunk_meta, call_meta, cidx, batches = _layout(cfg, cvar)
    nchunks = len(chunk_meta)

    # per-edge slot: chunk c = cidx[(tile, grp, rank//128)], partition rank%128
    cidx_arr = np.full((nt, ng, max(1, int(cvar.max()))), -1, dtype=np.int64)
    for (t, g, j), c in cidx.items():
        cidx_arr[t, g, j] = c
    t_s = (key_s // ng) % nt
    g_s = key_s % ng
    core_s = key_s // (ng * nt)
    j_s = rank // 128
    p_s = rank % 128
    c_s = cidx_arr[t_s, g_s, j_s]
    assert (c_s >= 0).all()

    wbase_arr = np.asarray(cfg.wbase, dtype=np.int64)
    val_s = (row_h[order] - wbase_arr[g_s % 2]).astype(np.int16)
    assert (val_s >= 0).all()

    # SWDGE queue q is served by Q7 cpu pair (2q, 2q+1), which read the index
    # data from partitions [32q, 32q+16) and [32q+16, 32q+32).  Assign calls
    # round-robin to queues and place each call's indices in its queue's
    # partition block (replicated for the pair's second cpu).
    nq = int(os.environ.get("KERNEL_QUEUES", "4"))
    queue_of_chunk = np.zeros(nchunks, dtype=np.int64)
    for k, (_bi, _b, c0, ncnt) in enumerate(call_meta):
        queue_of_chunk[c0 : c0 + ncnt] = k % nq
    q_s = queue_of_chunk[c_s]

    # int16 gather indices: element i of a call lives at
    # [32*q + i%16, chunk*8 + i//16]
    idx_all = np.zeros((cfg.ncores, 128, nchunks * 8), dtype=np.int16)
    idx_all[core_s, 32 * q_s + p_s % 16, c_s * 8 + p_s // 16] = val_s
    for q in range(nq):
        idx_all[:, 32 * q + 16 : 32 * q + 32, :] = idx_all[:, 32 * q : 32 * q + 16, :]

    mt_all = np.zeros((cfg.ncores, 128, nchunks * 128), dtype=ml_dtypes.bfloat16)
    mt_all[core_s, p_s, c_s * 128 + d_in_tile[order]] = norm[order].astype(
        ml_dtypes.bfloat16
    )
    return cvar, idx_all, mt_all


def _prep_inputs(cfg, K, idx_all, mt_all, x, lin_w, gcn_w, gamma, beta):
    import ml_dtypes

    bf = ml_dtypes.bfloat16
    x = np.asarray(x, dtype=np.float32)
    npl, npad = cfg.np_local, cfg.npad
    wlin = np.concatenate([lin_w[i].T for i in range(cfg.depth)], axis=1).astype(
        bf
    )  # [D, depth*D], column block i = lin_w[i].T  (k, o)
    wgcn = np.concatenate([gcn_w[i].T for i in range(cfg.depth)], axis=1).astype(bf)
    gb = np.stack(
        sum([[gamma[i], beta[i]] for i in range(cfg.depth)], []), axis=1
    ).astype(np.float32)  # [D, 2*depth]: columns g0,b0,g1,b1

    in_maps = []
    for r in range(cfg.ncores):
        xs = x[r * npl : (r + 1) * npl]
        xT = np.zeros((D, npad), dtype=bf)
        xT[:, :npl] = xs.T.astype(bf)
        in_maps.append(
            {
                "xT": np.ascontiguousarray(xT),
                "wlin": np.ascontiguousarray(wlin),
                "wgcn": np.ascontiguousarray(wgcn),
                "gb": np.ascontiguousarray(gb),
                "gidx": np.ascontiguousarray(idx_all[r]),
                "mt": np.ascontiguousarray(mt_all[r]),
            }
        )
    return in_maps


# ---------------------------------------------------------------- bass program
def _build_program(cfg, cvar):
    from concourse import bacc, mybir, tile

    f32 = mybir.dt.float32
    bf16 = mybir.dt.bfloat16
    i16 = mybir.dt.int16
    npl, npad, nt = cfg.np_local, cfg.npad, cfg.nt
    rg = [list(range(cfg.ncores))]
    inv_n = 1.0 / cfg.n_global
    skips = set(os.environ.get("KERNEL_SKIP", "").split(","))
    maxcalls = int(os.environ.get("KERNEL_MAXCALLS", "999999"))
    nq = int(os.environ.get("KERNEL_QUEUES", "4"))
    ncalls_done = 0

    chunk_meta, call_meta, cidx, batches = _layout(cfg, cvar)
    nchunks = len(chunk_meta)
    gcols = max(bc1 - bc0 for _t0, _t1, bc0, bc1 in batches) * 128

    nc = bacc.Bacc(
        "TRN2",
        target_bir_lowering=False,
        debug=False,
        num_devices=cfg.ncores,
        num_swdge_queues=nq,
    )

    xT = nc.dram_tensor("xT", [D, npad], bf16, kind="ExternalInput")
    wlin_d = nc.dram_tensor("wlin", [D, cfg.depth * D], bf16, kind="ExternalInput")
    wgcn_d = nc.dram_tensor("wgcn", [D, cfg.depth * D], bf16, kind="ExternalInput")
    gb_d = nc.dram_tensor("gb", [D, 2 * cfg.depth], f32, kind="ExternalInput")
    gidx_d = nc.dram_tensor("gidx", [128, nchunks * 8], i16, kind="ExternalInput")
    mt_d = nc.dram_tensor("mt", [128, nchunks * 128], bf16, kind="ExternalInput")
    outT_d = nc.dram_tensor("outT", [D, npl], bf16, kind="ExternalOutput")

    with tile.TileContext(nc) as tc:
        with (
            tc.tile_pool(name="big", bufs=1) as big,
            tc.tile_pool(name="gpool", bufs=2) as gpool,
            tc.tile_pool(name="mpool", bufs=2) as mpool,
            tc.tile_pool(name="ipool", bufs=2) as ipool,
            tc.tile_pool(name="cpool", bufs=4) as cpool,
            tc.tile_pool(name="small", bufs=1) as small,
            tc.tile_pool(name="stats", bufs=2) as stats_pool,
            tc.tile_pool(name="psA", bufs=4, space="PSUM") as psA,
            tc.tile_pool(name="psH", bufs=2, space="PSUM") as psH,
            tc.tile_pool(name="dram", bufs=1, space="DRAM") as dpool,
        ):
            hA = big.tile([128, npad], bf16)  # h (layer input), bf16
            hB = big.tile([128, npad], f32)  # accumulator hl+agg, f32
            consts = small.tile([128, 2], f32)  # col0 = 0.0, col1 = eps
            nc.vector.memset(consts[:, 0:1], 0.0)
            nc.vector.memset(consts[:, 1:2], float(cfg.eps))
            wlin = small.tile([128, cfg.depth * D], bf16)
            wgcn = small.tile([128, cfg.depth * D], bf16)
            gb = small.tile([128, 2 * cfg.depth], f32)
            sq_scr = small.tile([128, cfg.tb * 128], f32)  # Square scratch
            nbatch_tot = math.ceil(nt / cfg.tb)
            psums = small.tile([128, nbatch_tot], f32)
            psqs = small.tile([128, nbatch_tot], f32)

            nc.sync.dma_start(out=hA[:, :], in_=xT[:, :])
            nc.sync.dma_start(out=wlin[:, :], in_=wlin_d[:, :])
            nc.sync.dma_start(out=wgcn[:, :], in_=wgcn_d[:, :])
            nc.sync.dma_start(out=gb[:, :], in_=gb_d[:, :])

            cur, nxt = hA, hB
            for layer in range(cfg.depth):
                # ---- phase A: hg = h @ gcn_w.T (node-major, bf16) -> cc_in
                cc_in = dpool.tile([npl, D], bf16, name=f"cc_in_{layer}")
                cc_out = [
                    dpool.tile(
                        [cfg.nhalf, D],
                        bf16,
                        addr_space="Shared",
                        name=f"cc_out_{layer}_{h}",
                    )
                    for h in range(2)
                ]
                wg = wgcn[:, layer * D : (layer + 1) * D]
                GA = 8  # dest tiles per store group (one wide DMA each)
                for t0a in range(0, nt, GA):
                    t1a = min(nt, t0a + GA)
                    strip = cpool.tile(
                        [128, GA * 128], bf16, name="sb_hg", tag="sb_hg"
                    )
                    for t in range(t0a, t1a):
                        ps = psA.tile([128, 128], f32, name="ps_hg", tag="ps_hg")
                        nc.tensor.matmul(
                            ps[:, :],
                            lhsT=cur[:, t * 128 : (t + 1) * 128],
                            rhs=wg,
                            start=True,
                            stop=True,
                        )
                        nc.vector.tensor_copy(
                            strip[:, (t - t0a) * 128 : (t - t0a + 1) * 128], ps[:, :]
                        )
                    if "astore" in skips:
                        continue
                    n0 = t0a * 128
                    nfull = (min(npl, t1a * 128) - n0) // 128  # whole 128-row tiles
                    if nfull > 0:
                        nc.sync.dma_start(
                            out=cc_in[n0 : n0 + nfull * 128, :].rearrange(
                                "(t p) e -> p t e", p=128
                            ),
                            in_=strip[:, : nfull * 128].rearrange(
                                "p (t e) -> p t e", e=128
                            ),
                        )
                    rem = min(npl, t1a * 128) - (n0 + nfull * 128)
                    if rem > 0:
                        nc.sync.dma_start(
                            out=cc_in[n0 + nfull * 128 :, :],
                            in_=strip[:rem, nfull * 128 : (nfull + 1) * 128],
                        )
                    # phase B: trigger the half-AllGather as soon as its input
                    # rows are stored (half 0 mid-phase-A, half 1 at the end)
                    if "ag" not in skips:
                        for h in range(2):
                            if t0a * 128 < (h + 1) * cfg.half <= min(npl, t1a * 128):
                                nc.gpsimd.collective_compute(
                                    "AllGather",
                                    mybir.AluOpType.bypass,
                                    replica_groups=rg,
                                    ins=[
                                        cc_in[
                                            h * cfg.half : (h + 1) * cfg.half, :
                                        ].opt()
                                    ],
                                    outs=[cc_out[h][:, :].opt()],
                                )

                # ---- phase C: hl = h @ lin_w.T  (transposed layout, into nxt)
                wl = wlin[:, layer * D : (layer + 1) * D]
                if "hl" in skips:
                    nc.vector.memset(nxt[:, :], 0.0)
                c0 = 0
                while c0 < npad and "hl" not in skips:
                    cw = min(512, npad - c0)
                    ps = psH.tile([128, 512], f32, name="ps_hl", tag="ps_hl")
                    nc.tensor.matmul(
                        ps[:, :cw],
                        lhsT=wl,
                        rhs=cur[:, c0 : c0 + cw],
                        start=True,
                        stop=True,
                    )
                    nc.vector.tensor_copy(nxt[:, c0 : c0 + cw], ps[:, :cw])
                    c0 += cw

                # ---- phase D: bucketed dma_gather + one-hot matmul scatter-add
                skip_bn = "bn" in skips
                for bi_b, (t0, t1, bc0, bc1) in enumerate(batches):
                    nch_b = bc1 - bc0
                    g = gpool.tile([128, gcols], bf16, name="gbuf", tag="gbuf")
                    m = mpool.tile([128, gcols], bf16, name="mbuf", tag="mbuf")
                    wb = ipool.tile([128, gcols // 16], i16, name="wb", tag="wb")
                    nc.sync.dma_start(
                        out=wb[:, : nch_b * 8], in_=gidx_d[:, bc0 * 8 : bc1 * 8]
                    )
                    if "mt" not in skips:
                        nc.sync.dma_start(
                            out=m[:, : nch_b * 128],
                            in_=mt_d[:, bc0 * 128 : bc1 * 128],
                        )
                    if "gather" not in skips:
                        for callk, (bi2, gcall, c0g, ncall) in enumerate(call_meta):
                            if bi2 != bi_b:
                                continue
                            if ncalls_done >= maxcalls:
                                continue
                            ncalls_done += 1
                            cl0 = c0g - bc0
                            h, w = gcall // 2, gcall % 2
                            out3 = g[
                                :, cl0 * 128 : (cl0 + ncall) * 128
                            ].rearrange("p (c e) -> p c e", e=128)
                            q = callk % nq
                            nc.gpsimd.dma_gather(
                                out3,
                                cc_out[h][cfg.wbase[w] :, :],
                                wb[:, cl0 * 8 : (cl0 + ncall) * 8],
                                ncall * 128,
                                ncall * 128,
                                128,
                                single_packet=False,
                                queue_num=q,
                            )
                    if "aggmm" in skips:
                        continue
                    for t in range(t0, t1):
                        tchunks = [
                            cidx[(t, gg, j)]
                            for gg in range(cfg.ngroup)
                            for j in range(int(cvar[t, gg]))
                        ]
                        if not tchunks:
                            continue
                        ps = psA.tile([128, 128], f32, name="ps_agg", tag="ps_hg")
                        for si, c in enumerate(tchunks):
                            cl = c - bc0
                            nc.tensor.matmul(
                                ps[:, :],
                                lhsT=g[:, cl * 128 : (cl + 1) * 128],
                                rhs=m[:, cl * 128 : (cl + 1) * 128],
                                start=(si == 0),
                                stop=(si == len(tchunks) - 1),
                            )
                        nc.vector.tensor_add(
                            nxt[:, t * 128 : (t + 1) * 128],
                            nxt[:, t * 128 : (t + 1) * 128],
                            ps[:, :],
                        )
                    # per-batch partial BN stats (overlap with later batches)
                    if not skip_bn and "aggmm" not in skips:
                        lo, hi = t0 * 128, min(t1 * 128, npl)
                        nc.vector.reduce_sum(
                            out=psums[:, bi_b : bi_b + 1],
                            in_=nxt[:, lo:hi],
                            axis=mybir.AxisListType.X,
                        )
                        nc.scalar.activation(
                            out=sq_scr[:, : hi - lo],
                            in_=nxt[:, lo:hi],
                            func=mybir.ActivationFunctionType.Square,
                            bias=consts[:, 0:1],
                            accum_out=psqs[:, bi_b : bi_b + 1],
                        )

                # ---- phase E: combine partial stats + AllGather exchange
                st = stats_pool.tile([128, 2], f32, name=f"st_{layer}")
                if not skip_bn and "aggmm" not in skips:
                    nc.vector.reduce_sum(
                        out=st[:, 0:1], in_=psums[:, :], axis=mybir.AxisListType.X
                    )
                    nc.vector.reduce_sum(
                        out=st[:, 1:2], in_=psqs[:, :], axis=mybir.AxisListType.X
                    )
                else:
                    nc.vector.memset(st[:, :], 1.0)
                bn_in = dpool.tile([128, 2], f32, name=f"bn_in_{layer}")
                bn_out = dpool.tile(
                    [cfg.ncores * 128, 2],
                    f32,
                    addr_space="Shared",
                    name=f"bn_out_{layer}",
                )
                nc.sync.dma_start(out=bn_in[:, :], in_=st[:, :])
                gst = stats_pool.tile([128, 2], f32, name=f"gst_{layer}")
                if "ar" not in skips:
                    nc.gpsimd.collective_compute(
                        "AllGather",
                        mybir.AluOpType.bypass,
                        replica_groups=rg,
                        ins=[bn_in[:, :].opt()],
                        outs=[bn_out[:, :].opt()],
                    )
                    gst8 = stats_pool.tile([128, 2 * cfg.ncores], f32, name=f"g8_{layer}")
                    nc.sync.dma_start(
                        out=gst8[:, :].rearrange("p (c r) -> p c r", r=cfg.ncores),
                        in_=bn_out[:, :].rearrange("(r p) c -> p c r", p=128),
                    )
                    nc.vector.reduce_sum(
                        out=gst[:, :],
                        in_=gst8[:, :].rearrange("p (c r) -> p c r", r=cfg.ncores),
                        axis=mybir.AxisListType.X,
                    )
                else:
                    nc.vector.tensor_copy(gst[:, :], st[:, :])

                # scale = gamma * rsqrt(var+eps); bias = beta - mean*scale
                mu = stats_pool.tile([128, 1], f32, name=f"mu_{layer}")
                vr = stats_pool.tile([128, 1], f32, name=f"vr_{layer}")
                sc = stats_pool.tile([128, 1], f32, name=f"sc_{layer}")
                bi = stats_pool.tile([128, 1], f32, name=f"bi_{layer}")
                nc.scalar.mul(mu[:, :], gst[:, 0:1], inv_n)  # mean
                nc.vector.tensor_scalar(
                    out=vr[:, :],
                    in0=gst[:, 1:2],
                    scalar1=inv_n,
                    scalar2=None,
                    op0=mybir.AluOpType.mult,
                )
                mu2 = stats_pool.tile([128, 1], f32, name=f"mu2_{layer}")
                nc.scalar.activation(
                    out=mu2[:, :],
                    in_=mu[:, :],
                    func=mybir.ActivationFunctionType.Square,
                    bias=consts[:, 0:1],
                )
                nc.vector.tensor_sub(vr[:, :], vr[:, :], mu2[:, :])
                nc.scalar.activation(
                    out=vr[:, :],
                    in_=vr[:, :],
                    func=mybir.ActivationFunctionType.Sqrt,
                    bias=consts[:, 1:2],
                )
                nc.vector.reciprocal(vr[:, :], vr[:, :])  # rstd
                nc.vector.tensor_mul(sc[:, :], vr[:, :], gb[:, 2 * layer : 2 * layer + 1])
                nc.vector.tensor_mul(bi[:, :], mu[:, :], sc[:, :])
                nc.vector.tensor_sub(bi[:, :], gb[:, 2 * layer + 1 : 2 * layer + 2], bi[:, :])

                # ---- phase F: apply BN (+ReLU except last layer), into cur
                if "apply" in skips:
                    continue
                func = (
                    mybir.ActivationFunctionType.Relu
                    if layer != cfg.depth - 1
                    else mybir.ActivationFunctionType.Identity
                )
                # split so the next layer's phase A (or the final store) can
                # start on the first chunk while later chunks apply
                qsz = (nt // 4) * 128
                bounds = [0, qsz, 2 * qsz, 3 * qsz, npad]
                for c0f, c1f in zip(bounds[:-1], bounds[1:]):
                    nc.scalar.activation(
                        out=cur[:, c0f:c1f],
                        in_=nxt[:, c0f:c1f],
                        func=func,
                        bias=bi[:, :],
                        scale=sc[:, :],
                    )
                    if layer == cfg.depth - 1:
                        s1 = min(c1f, npl)
                        nc.sync.dma_start(
                            out=outT_d[:, c0f:s1], in_=cur[:, c0f:s1]
                        )

    nc.compile()
    return nc


# ---------------------------------------------------------------- entry points
def _make_runner(cfg, nc, in_maps):
    """Build a repeat-callable PJRT runner with device-resident inputs (no
    donation) for wall-clock timing. Returns (run_once, fetch_results)."""
    import jax
    from jax.experimental.shard_map import shard_map
    from jax.sharding import Mesh, NamedSharding, PartitionSpec

    from concourse import bass2jax, mybir

    bass2jax.install_neuronx_cc_hook()

    partition_name = nc.partition_id_tensor.name if nc.partition_id_tensor else None
    in_names, out_names, out_avals, zero_outs = [], [], [], []
    for alloc in nc.m.functions[0].allocations:
        if not isinstance(alloc, mybir.MemoryLocationSet):
            continue
        name = alloc.memorylocations[0].name
        if alloc.kind == "ExternalInput":
            if name != partition_name:
                in_names.append(name)
        elif alloc.kind == "ExternalOutput":
            out_names.append(name)
            shape = tuple(alloc.tensor_shape)
            dtype = mybir.dt.np(alloc.dtype)
            out_avals.append(jax.core.ShapedArray(shape, dtype))
            zero_outs.append(np.zeros(shape, dtype))
    n_params = len(in_names)
    all_in_names = list(in_names) + list(out_names)
    if partition_name is not None:
        all_in_names.append(partition_name)

    def _body(*args):
        operands = list(args)
        if partition_name is not None:
            operands.append(bass2jax.partition_id_tensor())
        outs = bass2jax._bass_exec_p.bind(
            *operands,
            out_avals=tuple(out_avals),
            in_names=tuple(all_in_names),
            out_names=tuple(out_names),
            lowering_input_output_aliases=(),
            sim_require_finite=True,
            sim_require_nnan=True,
            nc=nc,
        )
        return tuple(outs)

    n = cfg.ncores
    devices = jax.devices()[:n]
    mesh = Mesh(np.asarray(devices), ("core",))
    n_outs = len(out_names)
    in_specs = (PartitionSpec("core"),) * (n_params + n_outs)
    out_specs = (PartitionSpec("core"),) * n_outs
    sharded = jax.jit(
        shard_map(
            _body, mesh=mesh, in_specs=in_specs, out_specs=out_specs, check_rep=False
        ),
        keep_unused=True,
    )
    shd = NamedSharding(mesh, PartitionSpec("core"))
    concat_in = [
        jax.device_put(
            np.concatenate([np.asarray(in_maps[c][k]) for c in range(n)], axis=0), shd
        )
        for k in in_names
    ]
    concat_zeros = [
        jax.device_put(np.zeros((n * z.shape[0], *z.shape[1:]), z.dtype), shd)
        for z in zero_outs
    ]

    def run_once():
        outs = sharded(*concat_in, *concat_zeros)
        jax.block_until_ready(outs)
        return outs

    def fetch(outs):
        return [
            {
                k: np.asarray(outs[i]).reshape(n, *out_avals[i].shape)[c]
                for i, k in enumerate(out_names)
            }
            for c in range(n)
        ]

    return run_once, fetch


def _assemble(cfg, results):
    out = np.empty((cfg.n_global, D), dtype=np.float32)
    npl = cfg.np_local
    for r in range(cfg.ncores):
        out[r * npl : (r + 1) * npl] = results[r]["outT"].T.astype(np.float32)
    return out


def kernel(x, edge_index, edge_weight, lin_w, gcn_w, gamma, beta):
    global LAST_RUNNER
    cfg = _Cfg(N_GLOBAL, NCORES)
    x = np.asarray(x)
    assert x.shape == (cfg.n_global, D)
    K, idx_all, mt_all = _prep_graph(cfg, np.asarray(edge_index), np.asarray(edge_weight))
    in_maps = _prep_inputs(
        cfg, K, idx_all, mt_all, x, np.asarray(lin_w), np.asarray(gcn_w),
        np.asarray(gamma), np.asarray(beta),
    )
    nc = _build_program(cfg, K)
    run_once, fetch = _make_runner(cfg, nc, in_maps)
    LAST_RUNNER = (run_once, fetch)
    results = fetch(run_once())
    return _assemble(cfg, results)

